# revision 24
# baseline (speedup 1.0000x reference)
"""Multi-head attention block (B=8, S=1024, D=768, H=12) on 8 TRN2 NeuronCores.

Data-parallel: one batch element per core (attention is independent per batch).
Per-core pipeline (bf16 matmuls, fp32 PSUM accumulation):

  x [S,D] (bf16, host-cast) --PE-transpose--> xT [D,S]
  QT = Wq^T xT (+bq), KT = Wk^T xT (+bk)      [D,S]  (head pairs per 128-chunk)
  V  = x Wv (+bv)                             [S,D]  stored as [128,12,65] with
                                                      a ones column per head
  per head pair (2c,2c+1):
      S^T pair = K Q^T row-packed on PE array halves -> one [128,2048] PSUM tile
      exp(scale*S^T) -> PT pair [Sk, 2*Sq] (bf16, one ACT op)
      per head: O'^T = [V_h,1]^T PT (accum Sk) -> rows 0:64 = O^T, row 64 = rowsum
                OT_h = O'^T[0:64] * (1 / bcast(rowsum))
  out = O_cat @ Wp (+bp)                      [S,D]  (fp32 output)
"""

import numpy as np

B, S, DIM, H = 8, 1024, 768, 12
HD = DIM // H          # 64
SCALE = HD ** -0.5
N_CORES = 8
KC = DIM // 128        # 6 d-chunks
SC = S // 128          # 8 seq-chunks

_CACHE = {}


def _build():
    import concourse.mybir as mybir
    import concourse.tile as tile
    from concourse import bacc
    from concourse.masks import make_identity

    f32 = mybir.dt.float32
    bf16 = mybir.dt.bfloat16
    EXP = mybir.ActivationFunctionType.Exp

    nc = bacc.Bacc()

    x_ext = nc.declare_dram_parameter("x", [S, DIM], bf16, isOutput=False)
    Wq_ext = nc.declare_dram_parameter("Wq", [DIM, DIM], bf16, isOutput=False)
    bq_ext = nc.declare_dram_parameter("bq", [DIM], f32, isOutput=False)
    Wk_ext = nc.declare_dram_parameter("Wk", [DIM, DIM], bf16, isOutput=False)
    bk_ext = nc.declare_dram_parameter("bk", [DIM], f32, isOutput=False)
    Wv_ext = nc.declare_dram_parameter("Wv", [DIM, DIM], bf16, isOutput=False)
    bv_ext = nc.declare_dram_parameter("bv", [DIM], bf16, isOutput=False)
    Wp_ext = nc.declare_dram_parameter("Wp", [DIM, DIM], bf16, isOutput=False)
    bp_ext = nc.declare_dram_parameter("bp", [DIM], bf16, isOutput=False)
    out_ext = nc.declare_dram_parameter("out", [S, DIM], f32, isOutput=True)

    HALVES = ((0, 512), (512, 1024))
    VHALVES = ((0, 512), (512, DIM))

    with tile.TileContext(nc) as tc:
        with tc.tile_pool(name="persist", bufs=1) as sb, \
             tc.tile_pool(name="ps", bufs=1, space="PSUM") as ps:

            def p2(name, shape=(128, S), dtype=f32):
                return ps.tile(list(shape), dtype, tag="p2", bufs=2, name=name)

            # ---- constants ----
            ident = sb.tile([128, 128], bf16)
            make_identity(nc, ident)
            ones = sb.tile([1, 128], bf16)
            nc.vector.memset(ones, 1.0)
            # full-array ones block + zero-padded rowsum carrier: the softmax
            # broadcast matmul runs at K=128/M=128 so it never drags PE array
            # utilization below the HAM clock-gate threshold.
            ones2d = sb.tile([128, 128], bf16)
            nc.vector.memset(ones2d, 1.0)
            rs_z = sb.tile([128, S], bf16)
            nc.vector.memset(rs_z, 0.0)

            bq_sb = sb.tile([128, KC], f32)
            nc.sync.dma_start(out=bq_sb, in_=bq_ext[:].rearrange("(c p) -> p c", p=128))
            bk_sb = sb.tile([128, KC], f32)
            nc.sync.dma_start(out=bk_sb, in_=bk_ext[:].rearrange("(c p) -> p c", p=128))
            bv_row = sb.tile([1, DIM], bf16)
            nc.sync.dma_start(out=bv_row, in_=bv_ext[:].rearrange("(a d) -> a d", a=1))
            bp_row = sb.tile([1, DIM], bf16)
            nc.sync.dma_start(out=bp_row, in_=bp_ext[:].rearrange("(a d) -> a d", a=1))

            # x -> xT via PE transpose of 128x128 blocks
            xT = [sb.tile([128, S], bf16, name=f"xT{c}") for c in range(KC)]
            for c in range(KC):
                xt_ps = p2("xt_ps", (128, S), bf16)
                for s8 in range(SC):
                    xin = sb.tile([128, 128], bf16, tag="xin", bufs=8, name="xin")
                    nc.sync.dma_start(
                        out=xin,
                        in_=x_ext[s8 * 128:(s8 + 1) * 128, c * 128:(c + 1) * 128])
                    nc.tensor.transpose(xt_ps[:, s8 * 128:(s8 + 1) * 128], xin, ident)
                nc.vector.tensor_copy(xT[c], xt_ps)

            # broadcast bv/bp across 128 partitions: full-util K=128 matmul
            # against the zero-padded carrier (row 0 = bias, rows 1:128 = 0)
            # so even the warmup phase never drops PE utilization.
            bv_bc = sb.tile([128, DIM], f32)
            bp_bc = sb.tile([128, DIM], f32)
            for row, bc in ((bv_row, bv_bc), (bp_row, bp_bc)):
                nc.vector.tensor_copy(rs_z[0:1, 0:DIM], row[0:1, :])
                bc_ps = p2("bias_ps")
                for n0, n1 in VHALVES:
                    nc.tensor.matmul(bc_ps[:, n0:n1], ones2d,
                                     rs_z[:, n0:n1], start=True, stop=True)
                nc.scalar.copy(bc, bc_ps[:, 0:DIM])

            # persistent activation storage
            QT = [sb.tile([128, S], bf16, name=f"QT{c}") for c in range(KC)]
            KT = [sb.tile([128, S], bf16, name=f"KT{c}") for c in range(KC)]
            V12 = [sb.tile([128, H, HD + 1], bf16, name=f"V12_{s8}") for s8 in range(SC)]
            Wp_sb = [sb.tile([128, DIM], bf16, name=f"Wp{c}") for c in range(KC)]
            Wq_sb = [sb.tile([128, DIM], bf16, name=f"Wq{c}") for c in range(KC)]
            Wk_sb = [sb.tile([128, DIM], bf16, name=f"Wk{c}") for c in range(KC)]
            Wv_sb = [sb.tile([128, DIM], bf16, name=f"Wv{c}") for c in range(KC)]
            for c in range(KC):
                nc.sync.dma_start(out=Wq_sb[c], in_=Wq_ext[c * 128:(c + 1) * 128, :])
            for c in range(KC):
                nc.sync.dma_start(out=Wk_sb[c], in_=Wk_ext[c * 128:(c + 1) * 128, :])
            for c in range(KC):
                nc.sync.dma_start(out=Wv_sb[c], in_=Wv_ext[c * 128:(c + 1) * 128, :])
                nc.sync.dma_start(out=Wp_sb[c], in_=Wp_ext[c * 128:(c + 1) * 128, :])
            for s8 in range(SC):
                nc.vector.memset(V12[s8][:, :, HD:HD + 1], 1.0)

            # QT / KT chunk m: out[d_out, seq] = W^T @ xT, bias per partition
            def qk_chunk(W_sb, bias_sb, dst, m):
                q_ps = p2("q_ps")
                for k in range(KC):
                    for n0, n1 in HALVES:
                        nc.tensor.matmul(
                            q_ps[:, n0:n1],
                            W_sb[k][:, m * 128:(m + 1) * 128],
                            xT[k][:, n0:n1],
                            start=(k == 0), stop=(k == KC - 1))
                nc.vector.tensor_scalar_add(dst[m], q_ps, bias_sb[:, m:m + 1])

            # V natural layout: out[seq, d] = x @ Wv; +bv broadcast; ->bf16.
            # One seq-chunk at a time, trickled into pair 0's kc loop so the
            # matmuls fill the exp-paced PE slack; chunk kc lands just before
            # pair 0's PV needs it.
            def emit_v_chunk(s8):
                v_ps = p2("v_ps")
                for k in range(KC):
                    for n0, n1 in VHALVES:
                        nc.tensor.matmul(
                            v_ps[:, n0:n1],
                            xT[k][:, s8 * 128:(s8 + 1) * 128],
                            Wv_sb[k][:, n0:n1],
                            start=(k == 0), stop=(k == KC - 1))
                nc.vector.tensor_add(
                    V12[s8][:, :, 0:HD],
                    v_ps[:, 0:DIM].rearrange("p (h d) -> p h d", h=H),
                    bv_bc[:].rearrange("p (h d) -> p h d", h=H))

            # chunk 0 of Q and K first so head pair 0 can start immediately
            qk_chunk(Wq_sb, bq_sb, QT, 0)
            qk_chunk(Wk_sb, bk_sb, KT, 0)

            # ---- attention + output projection ----
            # Remaining QT/KT chunks are emitted inside the pair loop so the
            # scheduler starts attention as early as possible and fills the
            # exp-paced PE slack with projection matmuls.
            with tc.tile_pool(name="pb", bufs=1) as pb:
                OT = [pb.tile([128, S], bf16, name=f"OT{c}") for c in range(KC)]

                def norm_head(ov, c, half):
                    # rowsum (into row 0 of the zero-padded carrier) ->
                    # broadcast via full-util K=128 matmul -> 1/x -> normalize
                    nc.vector.tensor_copy(rs_z[0:1, :], ov[HD:HD + 1, :])
                    bc_ps = ps.tile([128, S], f32, tag="p2", bufs=2, name="bc_ps")
                    for n0, n1 in HALVES:
                        nc.tensor.matmul(bc_ps[:, n0:n1], ones2d,
                                         rs_z[:, n0:n1], start=True, stop=True)
                    rbc = pb.tile([HD, S], f32, tag="rbc", bufs=2, name="rbc")
                    nc.vector.reciprocal_approx_fast(rbc, bc_ps[0:HD, :])
                    base = half * HD
                    nc.vector.tensor_mul(OT[c][base:base + HD, :], ov[0:HD, :], rbc)

                def s_exp_pair(c):
                    # S^T for head pair (2c, 2c+1): row-packed matmuls on array
                    # halves run concurrently; exp'd scores buffer in SBUF.
                    pt = ([], [])
                    for kc in range(SC):
                        st_e = ps.tile([128, S], f32, tag="st2", bufs=2, name="st_e")
                        st_o = ps.tile([128, S], f32, tag="st2", bufs=2, name="st_o")
                        for n0, n1 in HALVES:
                            nc.tensor.matmul(
                                st_e[:, n0:n1],
                                KT[c][0:HD, kc * 128:(kc + 1) * 128],
                                QT[c][0:HD, n0:n1],
                                start=True, stop=True)
                            nc.tensor.matmul(
                                st_o[:, n0:n1],
                                KT[c][HD:128, kc * 128:(kc + 1) * 128],
                                QT[c][HD:128, n0:n1],
                                start=True, stop=True)
                        p_e = pb.tile([128, S], bf16, tag=f"pt{kc}e", bufs=2, name=f"pt{kc}e")
                        nc.scalar.activation(p_e, st_e, EXP, scale=SCALE)
                        p_o = pb.tile([128, S], bf16, tag=f"pt{kc}o", bufs=2, name=f"pt{kc}o")
                        nc.scalar.activation(p_o, st_o, EXP, scale=SCALE)
                        pt[0].append(p_e)
                        pt[1].append(p_o)
                        if c == 0:
                            emit_v_chunk(kc)
                    return pt

                def pv_norm(c, half, ptl):
                    ov = ps.tile([HD + 1, S], f32, tag="p2", bufs=2, name="ov")
                    for kc in range(SC):
                        for n0, n1 in HALVES:
                            nc.tensor.matmul(
                                ov[:, n0:n1],
                                V12[kc][:, 2 * c + half, :],
                                ptl[kc][:, n0:n1],
                                start=(kc == 0), stop=(kc == SC - 1))
                    norm_head(ov, c, half)

                # Half-skewed pipeline: pair c+1's S^T/exp is emitted between
                # the two PV halves of pair c, so ACT always has the next
                # pair's scores in flight while the PE drains PV.
                pts = s_exp_pair(0)
                # QK chunks prefetched two pairs ahead: chunk c+2 is emitted in
                # pair c, so by the time pair c+1's S^T is emitted its Q/K are
                # already computed and the exp stream never waits on them.
                qk_chunk(Wq_sb, bq_sb, QT, 1)
                qk_chunk(Wk_sb, bk_sb, KT, 1)
                for c in range(KC):
                    if c + 2 < KC:
                        qk_chunk(Wq_sb, bq_sb, QT, c + 2)
                        qk_chunk(Wk_sb, bk_sb, KT, c + 2)
                    pv_norm(c, 0, pts[0])
                    nxt = s_exp_pair(c + 1) if c + 1 < KC else None
                    pv_norm(c, 1, pts[1])
                    pts = nxt

                # out = O_cat @ Wp + bp
                for s8 in range(SC):
                    f_ps = p2("f_ps")
                    for k in range(KC):
                        for n0, n1 in VHALVES:
                            nc.tensor.matmul(
                                f_ps[:, n0:n1],
                                OT[k][:, s8 * 128:(s8 + 1) * 128],
                                Wp_sb[k][:, n0:n1],
                                start=(k == 0), stop=(k == KC - 1))
                    fin = pb.tile([128, DIM], f32, tag="fin", bufs=2, name="fin")
                    nc.vector.tensor_add(fin, f_ps[:, 0:DIM], bp_bc)
                    nc.sync.dma_start(out=out_ext[s8 * 128:(s8 + 1) * 128, :], in_=fin)

    nc.compile()
    return nc


def get_nc():
    if "nc" not in _CACHE:
        _CACHE["nc"] = _build()
    return _CACHE["nc"]


def kernel(x, Wq, bq, Wk, bk, Wv, bv, Wp, bp):
    import ml_dtypes
    from concourse.bass_utils import run_bass_kernel_spmd

    nc = get_nc()
    bfl = ml_dtypes.bfloat16
    x = np.ascontiguousarray(np.asarray(x, np.float32).astype(bfl))
    shared = {
        "Wq": np.ascontiguousarray(np.asarray(Wq, np.float32).astype(bfl)),
        "bq": np.ascontiguousarray(np.asarray(bq, np.float32)),
        "Wk": np.ascontiguousarray(np.asarray(Wk, np.float32).astype(bfl)),
        "bk": np.ascontiguousarray(np.asarray(bk, np.float32)),
        "Wv": np.ascontiguousarray(np.asarray(Wv, np.float32).astype(bfl)),
        "bv": np.ascontiguousarray(np.asarray(bv, np.float32).astype(bfl)),
        "Wp": np.ascontiguousarray(np.asarray(Wp, np.float32).astype(bfl)),
        "bp": np.ascontiguousarray(np.asarray(bp, np.float32).astype(bfl)),
    }
    in_maps = [{"x": x[b], **shared} for b in range(N_CORES)]
    res = run_bass_kernel_spmd(nc, in_maps, core_ids=list(range(N_CORES)))
    return np.stack([res.results[i]["out"] for i in range(N_CORES)], axis=0)
